# revision 4
# baseline (speedup 1.0000x reference)
"""Trainium2 Bass kernel for nn_BayesianDropoutLSTM_42468636623062.

Strategy (8 NeuronCores, data-parallel over batch):
  - Each core owns B/8 = 8 batch rows; weights replicated.
  - Phase A: embedding gather (indirect DMA) + px = xe @ W_ih.T + b_hh
    precompute (PE matmul, bias folded in as a rank-1 matmul), px -> HBM.
  - Phase B: 512 sequential LSTM steps. Per step, gates[8, 2048] accumulate
    in PSUM: px_t injected via an eye(8) matmul, then h_{t-1} @ W_hh.T as
    4x4 K-chunked matmuls (h.T chunks are the stationary operand).
    Gate order is permuted host-side to [i, f, o, g] so one ACT sigmoid
    covers i,f,o and one tanh covers g. c/h updates on DVE; h is
    PE-transposed back to [128, 8] chunks for the next step's matmuls.
  - Phase C: logits = hs @ fc_W.T + fc_b (PE, bias rank-1), then
    log_softmax = x - ln(sum(exp(x))) with exp/ln on ACT.
"""

import numpy as np

VOCAB, TAGS, EMB, HID = 100000, 48, 256, 512
B, S = 64, 512
H4 = 4 * HID
NCORES = 8
BL = B // NCORES  # 8 local batch rows per core

_CACHE = {}


def _build(nsteps=S, w_dt_name="float32", repeat=1):
    """Build + compile the per-core Bass program. Returns (nc, tok)."""
    import concourse.bass as bass
    import concourse.tile as tile
    from concourse import bacc, mybir
    from concourse.masks import make_identity
    from contextlib import ExitStack

    f32 = mybir.dt.float32
    i32 = mybir.dt.int32
    w_dt = getattr(mybir.dt, w_dt_name)
    AF = mybir.ActivationFunctionType
    OP = mybir.AluOpType

    tok = BL * nsteps
    ntiles = tok // 128
    assert tok % 128 == 0

    nc = bacc.Bacc(
        "TRN2",
        target_bir_lowering=False,
        debug=False,
        enable_asserts=True,
        num_devices=NCORES,
    )

    xidx = nc.dram_tensor("xidx", [128, ntiles], i32, kind="ExternalInput")
    emb = nc.dram_tensor("emb", [VOCAB, EMB], f32, kind="ExternalInput")
    wih = nc.dram_tensor("wih", [EMB, H4], f32, kind="ExternalInput")  # W_ih.T perm
    whh = nc.dram_tensor("whh", [HID, H4], w_dt, kind="ExternalInput")  # W_hh.T perm
    bhh = nc.dram_tensor("bhh", [1, H4], f32, kind="ExternalInput")
    fcw = nc.dram_tensor("fcw", [HID, TAGS], f32, kind="ExternalInput")  # fc_W.T
    fcb = nc.dram_tensor("fcb", [1, TAGS], f32, kind="ExternalInput")
    outd = nc.dram_tensor("out", [tok, TAGS], f32, kind="ExternalOutput")

    with tile.TileContext(nc) as tc, ExitStack() as ctx:
        const_pool = ctx.enter_context(tc.tile_pool(name="const", bufs=1))
        dram_pool = ctx.enter_context(tc.tile_pool(name="dram", bufs=1, space="DRAM"))

        px_dram = dram_pool.tile([tok, H4], w_dt)
        hs_dram = dram_pool.tile([tok, HID], f32)

        id128 = const_pool.tile([128, 128], f32)
        make_identity(nc, id128[:])
        ones_r = const_pool.tile([1, 128], f32)
        nc.vector.memset(ones_r[:], 1.0)
        x_sb = const_pool.tile([128, ntiles], i32)
        nc.sync.dma_start(x_sb[:], xidx[:])
        bhh_sb = const_pool.tile([1, H4], f32)
        nc.sync.dma_start(bhh_sb[:], bhh[:])
        fcb_sb = const_pool.tile([1, TAGS], f32)
        nc.sync.dma_start(fcb_sb[:], fcb[:])
        eye8 = id128[:8, :8]
        if w_dt != f32:
            eye8w_t = const_pool.tile([8, 8], w_dt)
            nc.vector.tensor_copy(eye8w_t[:], eye8)
            eye8w = eye8w_t[:]
        else:
            eye8w = eye8

        wih_sb = []
        for j in range(EMB // 128):
            t = const_pool.tile([128, H4], f32, tag=f"wih{j}")
            nc.sync.dma_start(t[:], wih[j * 128 : (j + 1) * 128, :])
            wih_sb.append(t)
        whh_sb = []
        for j in range(HID // 128):
            t = const_pool.tile([128, H4], w_dt, tag=f"whh{j}")
            nc.sync.dma_start(t[:], whh[j * 128 : (j + 1) * 128, :])
            whh_sb.append(t)
        fcw_sb = []
        for j in range(HID // 128):
            t = const_pool.tile([128, TAGS], f32, tag=f"fcw{j}")
            nc.sync.dma_start(t[:], fcw[j * 128 : (j + 1) * 128, :])
            fcw_sb.append(t)

        px_v = px_dram[:].rearrange("(b s) g -> b s g", b=BL)
        hs_v = hs_dram[:].rearrange("(b s) h -> b s h", b=BL)

        for _rep in range(repeat):
            # ---------------- Phase A: gather + px precompute ----------------
            with tc.tile_pool(name="pa_sb", bufs=3) as pa, tc.tile_pool(
                name="pa_ps", bufs=2, space="PSUM"
            ) as pa_ps, tc.tile_pool(name="pa_pxps", bufs=1, space="PSUM") as pa_pxps:
                for k in range(ntiles):
                    xe = pa.tile([128, EMB], f32, tag="xe")
                    nc.gpsimd.indirect_dma_start(
                        out=xe[:],
                        out_offset=None,
                        in_=emb[:],
                        in_offset=bass.IndirectOffsetOnAxis(
                            ap=x_sb[:, k : k + 1], axis=0
                        ),
                    )
                    xeT = []
                    for j in range(EMB // 128):
                        tp = pa_ps.tile([128, 128], f32, tag="trps")
                        nc.tensor.transpose(
                            tp[:], xe[:, j * 128 : (j + 1) * 128], id128[:]
                        )
                        xt = pa.tile([128, 128], f32, tag=f"xeT{j}")
                        nc.vector.tensor_copy(xt[:], tp[:])
                        xeT.append(xt)
                    pxps = pa_pxps.tile([128, H4], f32, tag="pxps")
                    for bank in range(4):
                        bs = slice(bank * 512, (bank + 1) * 512)
                        for j in range(EMB // 128):
                            nc.tensor.matmul(
                                pxps[:, bs],
                                lhsT=xeT[j][:],
                                rhs=wih_sb[j][:, bs],
                                start=(j == 0),
                                stop=False,
                            )
                        nc.tensor.matmul(
                            pxps[:, bs],
                            lhsT=ones_r[:1, :],
                            rhs=bhh_sb[:1, bs],
                            start=False,
                            stop=True,
                        )
                    px_sb = pa.tile([128, H4], w_dt, tag="px_sb")
                    nc.vector.tensor_copy(px_sb[:], pxps[:])
                    nc.sync.dma_start(px_dram[k * 128 : (k + 1) * 128, :], px_sb[:])

            # ---------------- Phase B: recurrence ----------------
            with tc.tile_pool(name="pb_state", bufs=1) as pst, tc.tile_pool(
                name="pb_hT", bufs=10
            ) as phT, tc.tile_pool(name="pb_px", bufs=4) as ppx, tc.tile_pool(
                name="pb_wk", bufs=2
            ) as pwk, tc.tile_pool(
                name="pb_ps", bufs=1, space="PSUM"
            ) as pps, tc.tile_pool(
                name="pb_hps", bufs=2, space="PSUM"
            ) as phps:
                c_sb = pst.tile([8, HID], f32)
                nc.vector.memset(c_sb[:], 0.0)
                hT = []
                for j in range(4):
                    t = phT.tile([128, 8], w_dt, tag="hT")
                    nc.vector.memset(t[:], 0.0)
                    hT.append(t)

                for t_ in range(nsteps):
                    px_t = ppx.tile([8, H4], w_dt, tag="px_t")
                    nc.sync.dma_start(px_t[:], px_v[:, t_, :])
                    gps = pps.tile([8, H4], f32, tag="gps")
                    for bank in range(4):
                        bs = slice(bank * 512, (bank + 1) * 512)
                        nc.tensor.matmul(
                            gps[:, bs],
                            lhsT=eye8w,
                            rhs=px_t[:, bs],
                            start=True,
                            stop=False,
                        )
                        for j in range(4):
                            nc.tensor.matmul(
                                gps[:, bs],
                                lhsT=hT[j][:],
                                rhs=whh_sb[j][:, bs],
                                start=False,
                                stop=(j == 3),
                            )
                    sig = pwk.tile([8, 1536], f32, tag="sig")
                    nc.scalar.activation(sig[:], gps[:, 0:1536], AF.Sigmoid)
                    g_t = pwk.tile([8, 512], f32, tag="g_t")
                    nc.scalar.activation(g_t[:], gps[:, 1536:2048], AF.Tanh)
                    t1 = pwk.tile([8, 512], f32, tag="t1")
                    nc.vector.tensor_tensor(
                        out=t1[:], in0=sig[:, 0:512], in1=g_t[:], op=OP.mult
                    )
                    t2 = pwk.tile([8, 512], f32, tag="t2")
                    nc.vector.tensor_tensor(
                        out=t2[:], in0=sig[:, 512:1024], in1=c_sb[:], op=OP.mult
                    )
                    nc.vector.tensor_tensor(
                        out=c_sb[:], in0=t1[:], in1=t2[:], op=OP.add
                    )
                    tc_t = pwk.tile([8, 512], f32, tag="tc_t")
                    nc.scalar.activation(tc_t[:], c_sb[:], AF.Tanh)
                    h_t = pwk.tile([8, 512], f32, tag="h_t")
                    nc.vector.tensor_tensor(
                        out=h_t[:], in0=sig[:, 1024:1536], in1=tc_t[:], op=OP.mult
                    )
                    hps = phps.tile([128, 32], f32, tag="hps")
                    new_hT = []
                    for j in range(4):
                        nc.tensor.transpose(
                            hps[:, j * 8 : (j + 1) * 8],
                            h_t[:, j * 128 : (j + 1) * 128],
                            eye8,
                        )
                        nt = phT.tile([128, 8], w_dt, tag="hT")
                        if j % 2 == 0:
                            nc.scalar.copy(nt[:], hps[:, j * 8 : (j + 1) * 8])
                        else:
                            nc.vector.tensor_copy(nt[:], hps[:, j * 8 : (j + 1) * 8])
                        new_hT.append(nt)
                    hT = new_hT
                    nc.sync.dma_start(hs_v[:, t_, :], h_t[:])

            # ---------------- Phase C: FC + log_softmax ----------------
            with tc.tile_pool(name="pc_sb", bufs=3) as pc, tc.tile_pool(
                name="pc_keep", bufs=1
            ) as pck, tc.tile_pool(name="pc_ps", bufs=2, space="PSUM") as pc_ps, tc.tile_pool(
                name="pc_lps", bufs=2, space="PSUM"
            ) as pc_lps:
                logit_sb = pck.tile([128, ntiles * TAGS], f32)
                e_sb = pck.tile([128, ntiles * TAGS], f32)
                for k in range(ntiles):
                    hsk = pc.tile([128, HID], f32, tag="hsk")
                    nc.sync.dma_start(hsk[:], hs_dram[k * 128 : (k + 1) * 128, :])
                    lps = pc_lps.tile([128, TAGS], f32, tag="lps")
                    for j in range(4):
                        tp = pc_ps.tile([128, 128], f32, tag="trps")
                        nc.tensor.transpose(
                            tp[:], hsk[:, j * 128 : (j + 1) * 128], id128[:]
                        )
                        hsT = pc.tile([128, 128], f32, tag="hsT")
                        nc.vector.tensor_copy(hsT[:], tp[:])
                        nc.tensor.matmul(
                            lps[:],
                            lhsT=hsT[:],
                            rhs=fcw_sb[j][:],
                            start=(j == 0),
                            stop=False,
                        )
                    nc.tensor.matmul(
                        lps[:],
                        lhsT=ones_r[:1, :],
                        rhs=fcb_sb[:1, :],
                        start=False,
                        stop=True,
                    )
                    ks = slice(k * TAGS, (k + 1) * TAGS)
                    nc.scalar.activation(e_sb[:, ks], lps[:], AF.Exp)
                    nc.vector.tensor_copy(logit_sb[:, ks], lps[:])
                ssum = pck.tile([128, ntiles], f32)
                ev = e_sb[:].rearrange("p (k t) -> p k t", t=TAGS)
                nc.vector.tensor_reduce(
                    out=ssum[:], in_=ev, axis=mybir.AxisListType.X, op=OP.add
                )
                lsum = pck.tile([128, ntiles], f32)
                nc.scalar.activation(lsum[:], ssum[:], AF.Ln)
                for k in range(ntiles):
                    ks = slice(k * TAGS, (k + 1) * TAGS)
                    o_sb = pc.tile([128, TAGS], f32, tag="o_sb")
                    nc.vector.tensor_scalar(
                        out=o_sb[:],
                        in0=logit_sb[:, ks],
                        scalar1=lsum[:, k : k + 1],
                        scalar2=None,
                        op0=OP.subtract,
                    )
                    nc.sync.dma_start(outd[k * 128 : (k + 1) * 128, :], o_sb[:])

    nc.compile()
    return nc, tok


def _prep_inputs(x, emb, W_ih, W_hh, b_hh, fc_W, fc_b, nsteps, w_dt_name):
    x = np.asarray(x)
    emb = np.ascontiguousarray(np.asarray(emb, dtype=np.float32))
    W_ih = np.asarray(W_ih, dtype=np.float32)
    W_hh = np.asarray(W_hh, dtype=np.float32)
    b_hh = np.asarray(b_hh, dtype=np.float32)
    fc_W = np.asarray(fc_W, dtype=np.float32)
    fc_b = np.asarray(fc_b, dtype=np.float32)

    w_np = np.float32 if w_dt_name == "float32" else np.dtype("bfloat16")
    try:
        import ml_dtypes

        if w_dt_name == "bfloat16":
            w_np = ml_dtypes.bfloat16
    except ImportError:
        pass

    perm = np.r_[0:512, 512:1024, 1536:2048, 1024:1536]  # -> [i, f, o, g]
    wih_p = np.ascontiguousarray(W_ih[perm, :].T, dtype=np.float32)
    whh_p = np.ascontiguousarray(W_hh[perm, :].T).astype(w_np)
    bhh_p = np.ascontiguousarray(b_hh[perm].reshape(1, H4), dtype=np.float32)
    fcw_t = np.ascontiguousarray(fc_W.T, dtype=np.float32)
    fcb_r = np.ascontiguousarray(fc_b.reshape(1, TAGS), dtype=np.float32)

    tok = BL * nsteps
    in_maps = []
    for c in range(NCORES):
        xc = x[c * BL : (c + 1) * BL, :nsteps].astype(np.int32).reshape(tok)
        xdev = np.ascontiguousarray(xc.reshape(tok // 128, 128).T)
        in_maps.append(
            {
                "xidx": xdev,
                "emb": emb,
                "wih": wih_p,
                "whh": whh_p,
                "bhh": bhh_p,
                "fcw": fcw_t,
                "fcb": fcb_r,
            }
        )
    return in_maps


def _get_runner(nsteps=S, w_dt_name="float32", repeat=1):
    """Returns (run_fn, nc). run_fn(in_maps) -> list of per-core {name: arr}."""
    key = (nsteps, w_dt_name, repeat)
    if key in _CACHE:
        return _CACHE[key]

    import jax
    from jax.sharding import Mesh, PartitionSpec
    from jax.experimental.shard_map import shard_map
    from concourse import bass2jax, mybir

    nckey = ("nc", nsteps, w_dt_name) if repeat == 1 else ("ncr", nsteps, w_dt_name, repeat)
    if nckey not in _CACHE:
        _CACHE[nckey] = _build(nsteps, w_dt_name, repeat)
    nc, tok = _CACHE[nckey]
    bass2jax.install_neuronx_cc_hook()

    partition_name = nc.partition_id_tensor.name if nc.partition_id_tensor else None
    in_names, out_names, out_avals, zero_shapes = [], [], [], []
    for alloc in nc.m.functions[0].allocations:
        if not isinstance(alloc, mybir.MemoryLocationSet):
            continue
        name = alloc.memorylocations[0].name
        if alloc.kind == "ExternalInput":
            if name != partition_name:
                in_names.append(name)
        elif alloc.kind == "ExternalOutput":
            shape = tuple(alloc.tensor_shape)
            dtype = mybir.dt.np(alloc.dtype)
            out_names.append(name)
            out_avals.append(jax.core.ShapedArray(shape, dtype))
            zero_shapes.append((shape, dtype))
    n_params = len(in_names)
    n_outs = len(out_avals)
    all_in_names = in_names + out_names + ([partition_name] if partition_name else [])
    donate = tuple(range(n_params, n_params + n_outs))

    def _body(*args):
        operands = list(args)
        if partition_name is not None:
            operands.append(bass2jax.partition_id_tensor())
        return tuple(
            bass2jax._bass_exec_p.bind(
                *operands,
                out_avals=tuple(out_avals),
                in_names=tuple(all_in_names),
                out_names=tuple(out_names),
                lowering_input_output_aliases=(),
                sim_require_finite=True,
                sim_require_nnan=True,
                nc=nc,
            )
        )

    devices = jax.devices()[:NCORES]
    mesh = Mesh(np.asarray(devices), ("core",))
    sharded = jax.jit(
        shard_map(
            _body,
            mesh=mesh,
            in_specs=(PartitionSpec("core"),) * (n_params + n_outs),
            out_specs=(PartitionSpec("core"),) * n_outs,
            check_rep=False,
        ),
        donate_argnums=donate,
        keep_unused=True,
    )

    def run_fn(in_maps):
        concat_in = [
            np.concatenate([np.asarray(m[nm]) for m in in_maps], axis=0)
            for nm in in_names
        ]
        concat_zeros = [
            np.zeros((NCORES * s[0], *s[1:]), d) for (s, d) in zero_shapes
        ]
        out_arrs = sharded(*concat_in, *concat_zeros)
        jax.block_until_ready(out_arrs)
        return [
            {
                nm: np.asarray(out_arrs[i]).reshape(NCORES, *out_avals[i].shape)[c]
                for i, nm in enumerate(out_names)
            }
            for c in range(NCORES)
        ]

    _CACHE[key] = (run_fn, nc)
    return run_fn, nc


W_DT_NAME = "float32"


def kernel(x, emb, W_ih, W_hh, b_hh, fc_W, fc_b):
    from concourse.bass_utils import run_bass_kernel_spmd

    key = ("nc", S, W_DT_NAME)
    if key not in _CACHE:
        _CACHE[key] = _build(S, W_DT_NAME, 1)
    nc, _tok = _CACHE[key]
    in_maps = _prep_inputs(x, emb, W_ih, W_hh, b_hh, fc_W, fc_b, S, W_DT_NAME)
    res = run_bass_kernel_spmd(nc, in_maps, core_ids=list(range(NCORES)))
    out = np.stack(
        [res.results[c]["out"].reshape(BL, S, TAGS) for c in range(NCORES)]
    )
    return out.reshape(B, S, TAGS).astype(np.float32)


# revision 17
# speedup vs baseline: 3579.2854x; 3579.2854x over previous
"""Trainium2 Bass kernel for nn_BayesianDropoutLSTM_42468636623062.

Strategy (8 NeuronCores, data-parallel over batch):
  - Each core owns B/8 = 8 batch rows; weights replicated.
  - Phase A: embedding gather (indirect DMA) + px = xe @ W_ih.T + b_hh
    precompute (PE matmul, bias folded in as a rank-1 matmul), px -> HBM.
  - Phase B: 512 sequential LSTM steps. The 2048 gate columns are permuted
    host-side into hidden-chunk-major order [i_c|f_c|o_c|g_c] per chunk so
    each chunk's sigmoid is one ACT op and chunks pipeline: chunk c's
    elementwise chain overlaps the PE matmuls of the other chunk / next
    step. gates accumulate in PSUM: px_t injected via an eye(8) matmul,
    then h_{t-1} @ W_hh.T as K-chunked matmuls with h.T as the stationary
    operand (h is PE-transposed back to [128, 8] chunks each step).
    All big matmuls use float32r (1 cycle/row vs 4 for fp32).
  - Phase C: logits = hs @ fc_W.T + fc_b (PE, bias rank-1), then
    log_softmax = x - ln(sum(exp(x))) with exp/ln on ACT.
"""

import numpy as np

VOCAB, TAGS, EMB, HID = 100000, 48, 256, 512
B, S = 64, 512
H4 = 4 * HID
NCORES = 8
BL = B // NCORES  # 8 local batch rows per core

NCHUNKS = 4  # hidden-dim pipeline chunks in phase B (1, 2, or 4)

_CACHE = {}


def _build(nsteps=S, w_dt_name="float32", repeat=1, nchunks=NCHUNKS, abl=(),
           compile_nc=True):
    """Build + compile the per-core Bass program. Returns (nc, tok)."""
    abl = set(abl)
    import concourse.bass as bass
    import concourse.tile as tile
    from concourse import bacc, mybir
    from concourse.masks import make_identity
    from contextlib import ExitStack

    f32 = mybir.dt.float32
    f32r = mybir.dt.float32r
    i32 = mybir.dt.int32
    w_dt = getattr(mybir.dt, w_dt_name)
    AF = mybir.ActivationFunctionType
    OP = mybir.AluOpType

    # matmul storage dtype for the big MMs: float32r streams 1 cycle/row on
    # the PE (vs 4 for fp32) at N>=256. The BIR verifier requires f32r matmul
    # operands to be produced rounded, so the feeding tiles are natively f32r
    # and weights are converted once on-device via DVE copies.
    mm_dt = f32r if w_dt == f32 else w_dt

    tok = BL * nsteps
    ntiles = tok // 128
    assert tok % 128 == 0
    W = HID // nchunks       # hidden units per chunk
    GW = 4 * W               # gate columns per chunk
    KCH = HID // 128         # K chunks (4)
    KPC = KCH // nchunks     # K chunks produced per hidden chunk

    nc = bacc.Bacc(
        "TRN2",
        target_bir_lowering=False,
        debug=False,
        enable_asserts=True,
        num_devices=NCORES,
    )

    xidx = nc.dram_tensor("xidx", [128, ntiles], i32, kind="ExternalInput")
    emb = nc.dram_tensor("emb", [VOCAB, EMB], f32, kind="ExternalInput")
    wih = nc.dram_tensor("wih", [EMB, H4], f32, kind="ExternalInput")  # W_ih.T perm
    whh = nc.dram_tensor("whh", [HID, H4], w_dt, kind="ExternalInput")  # W_hh.T perm
    bhh = nc.dram_tensor("bhh", [1, H4], f32, kind="ExternalInput")
    fcw = nc.dram_tensor("fcw", [HID, TAGS], f32, kind="ExternalInput")  # fc_W.T
    fcb = nc.dram_tensor("fcb", [1, TAGS], f32, kind="ExternalInput")
    outd = nc.dram_tensor("out", [tok, TAGS], f32, kind="ExternalOutput")

    with tile.TileContext(nc) as tc, ExitStack() as ctx:
        const_pool = ctx.enter_context(tc.tile_pool(name="const", bufs=1))
        dram_pool = ctx.enter_context(tc.tile_pool(name="dram", bufs=1, space="DRAM"))

        px_dram = dram_pool.tile([tok, H4], mm_dt)
        hs_dram = dram_pool.tile([tok, HID], f32)

        id128 = const_pool.tile([128, 128], f32)
        make_identity(nc, id128[:])
        ones_r = const_pool.tile([1, 128], f32)
        nc.vector.memset(ones_r[:], 1.0)
        x_sb = const_pool.tile([128, ntiles], i32)
        nc.sync.dma_start(x_sb[:], xidx[:])
        bhh_sb = const_pool.tile([1, H4], f32)
        nc.sync.dma_start(bhh_sb[:], bhh[:])
        fcb_sb = const_pool.tile([1, TAGS], f32)
        nc.sync.dma_start(fcb_sb[:], fcb[:])
        eye8 = id128[:8, :8]
        eye8w_t = const_pool.tile([8, 8], mm_dt)
        nc.vector.tensor_copy(eye8w_t[:], eye8)
        eye8w = eye8w_t[:]
        ones_rr = const_pool.tile([1, 128], mm_dt)
        nc.vector.tensor_copy(ones_rr[:], ones_r[:])
        bhh_rr = const_pool.tile([1, H4], mm_dt)
        nc.vector.tensor_copy(bhh_rr[:], bhh_sb[:])

        wih_sb = []
        whh_sb = []
        with tc.tile_pool(name="wstage", bufs=2) as pstage:
            for j in range(EMB // 128):
                st = pstage.tile([128, H4], f32, tag="st")
                nc.sync.dma_start(st[:], wih[j * 128 : (j + 1) * 128, :])
                t = const_pool.tile([128, H4], mm_dt, tag=f"wih{j}")
                nc.vector.tensor_copy(t[:], st[:])
                wih_sb.append(t)
            for j in range(HID // 128):
                if w_dt == f32:
                    st = pstage.tile([128, H4], f32, tag="st")
                    nc.sync.dma_start(st[:], whh[j * 128 : (j + 1) * 128, :])
                    t = const_pool.tile([128, H4], mm_dt, tag=f"whh{j}")
                    nc.vector.tensor_copy(t[:], st[:])
                else:
                    t = const_pool.tile([128, H4], w_dt, tag=f"whh{j}")
                    nc.sync.dma_start(t[:], whh[j * 128 : (j + 1) * 128, :])
                whh_sb.append(t)
        fcw_sb = []
        for j in range(HID // 128):
            t = const_pool.tile([128, TAGS], f32, tag=f"fcw{j}")
            nc.sync.dma_start(t[:], fcw[j * 128 : (j + 1) * 128, :])
            fcw_sb.append(t)

        px_v = px_dram[:].rearrange("(b s) g -> b s g", b=BL)
        hs_v = hs_dram[:].rearrange("(b s) h -> b s h", b=BL)

        for _rep in range(repeat):
            # ---------------- Phase A: gather + px precompute ----------------
            with tc.tile_pool(name="pa_sb", bufs=3) as pa, tc.tile_pool(
                name="pa_ps", bufs=2, space="PSUM"
            ) as pa_ps, tc.tile_pool(name="pa_pxps", bufs=1, space="PSUM") as pa_pxps:
                for k in range(ntiles):
                    xe = pa.tile([128, EMB], f32, tag="xe")
                    nc.gpsimd.indirect_dma_start(
                        out=xe[:],
                        out_offset=None,
                        in_=emb[:],
                        in_offset=bass.IndirectOffsetOnAxis(
                            ap=x_sb[:, k : k + 1], axis=0
                        ),
                    )
                    xeT = []
                    for j in range(EMB // 128):
                        tp = pa_ps.tile([128, 128], f32, tag="trps")
                        nc.tensor.transpose(
                            tp[:], xe[:, j * 128 : (j + 1) * 128], id128[:]
                        )
                        xt = pa.tile([128, 128], mm_dt, tag=f"xeT{j}")
                        nc.vector.tensor_copy(xt[:], tp[:])
                        xeT.append(xt)
                    pxps = pa_pxps.tile([128, H4], f32, tag="pxps")
                    for bank in range(4):
                        bs = slice(bank * 512, (bank + 1) * 512)
                        for j in range(EMB // 128):
                            nc.tensor.matmul(
                                pxps[:, bs],
                                lhsT=xeT[j][:],
                                rhs=wih_sb[j][:, bs],
                                start=(j == 0),
                                stop=False,
                            )
                        nc.tensor.matmul(
                            pxps[:, bs],
                            lhsT=ones_rr[:1, :],
                            rhs=bhh_rr[:1, bs],
                            start=False,
                            stop=True,
                        )
                    px_sb = pa.tile([128, H4], mm_dt, tag="px_sb")
                    nc.vector.tensor_copy(px_sb[:], pxps[:])
                    nc.sync.dma_start(px_dram[k * 128 : (k + 1) * 128, :], px_sb[:])

            # ---------------- Phase B: recurrence ----------------
            with tc.tile_pool(name="pb_state", bufs=1) as pst, tc.tile_pool(
                name="pb_hT", bufs=10
            ) as phT, tc.tile_pool(name="pb_px", bufs=2) as ppx, tc.tile_pool(
                name="pb_wk", bufs=3
            ) as pwk, tc.tile_pool(
                name="pb_ps", bufs={1: 1, 2: 3, 4: 6}[nchunks],
                space="PSUM",
            ) as pps, tc.tile_pool(
                name="pb_hps", bufs=2, space="PSUM"
            ) as phps:
                c_sb = pst.tile([8, HID], f32)
                nc.vector.memset(c_sb[:], 0.0)
                KCHn = HID // 128
                hT = []
                for j in range(KCHn):
                    t = phT.tile([128, 8], mm_dt, tag="hT")
                    mv = t[:].bitcast(f32) if mm_dt == f32r else t[:]
                    nc.vector.memset(mv, 0.0)
                    hT.append(t)

                nbank = GW // 512  # PSUM banks per chunk psum tile
                TB = 4  # steps per batched px-load / hs-store DMA
                assert nsteps % TB == 0
                px_t8 = None
                hbuf = None
                for t_ in range(nsteps):
                    ti = t_ % TB
                    if ti == 0:
                        px_t8 = ppx.tile([8, TB * H4], mm_dt, tag="px_t")
                        nc.sync.dma_start(
                            px_t8[:].rearrange("b (s g) -> b s g", s=TB),
                            px_v[:, t_ : t_ + TB, :],
                        )
                        hbuf = pwk.tile([8, TB * HID], f32, tag="h_t")
                    h_t = hbuf[:, ti * HID : (ti + 1) * HID]
                    new_hT = [None] * KCHn
                    hps = phps.tile([128, 8 * KCHn], f32, tag="hps")
                    for cki in range(nchunks):
                        gps = pps.tile([8, GW], f32, tag="gps")
                        for bank in range(nbank):
                            bs = slice(bank * 512, (bank + 1) * 512)
                            bsg = slice(
                                cki * GW + bank * 512, cki * GW + (bank + 1) * 512
                            )
                            bsg_px = slice(
                                ti * H4 + cki * GW + bank * 512,
                                ti * H4 + cki * GW + (bank + 1) * 512,
                            )
                            nc.tensor.matmul(
                                gps[:, bs],
                                lhsT=eye8w,
                                rhs=px_t8[:, bsg_px],
                                start=True,
                                stop=False,
                            )
                            if "no_hmm" in abl:
                                continue
                            for j in range(KCHn):
                                nc.tensor.matmul(
                                    gps[:, bs],
                                    lhsT=hT[j][:],
                                    rhs=whh_sb[j][:, bsg],
                                    start=False,
                                    stop=(j == KCHn - 1),
                                )
                        # chunk gate layout: [i (W) | f (W) | o (W) | g (W)]
                        sig = pwk.tile([8, 3 * W], f32, tag="sig")
                        nc.scalar.activation(sig[:], gps[:, 0 : 3 * W], AF.Sigmoid)
                        g_t = pwk.tile([8, W], f32, tag="g_t")
                        nc.scalar.activation(g_t[:], gps[:, 3 * W : 4 * W], AF.Tanh)
                        ch = slice(cki * W, (cki + 1) * W)  # hidden slice
                        t1 = pwk.tile([8, W], f32, tag="t1")
                        nc.vector.tensor_tensor(
                            out=t1[:], in0=sig[:, 0:W], in1=g_t[:], op=OP.mult
                        )
                        t2 = pwk.tile([8, W], f32, tag="t2")
                        eng_t2 = nc.gpsimd if "gp_t2" in abl else nc.vector
                        eng_t2.tensor_tensor(
                            out=t2[:],
                            in0=sig[:, W : 2 * W],
                            in1=c_sb[:, ch],
                            op=OP.mult,
                        )
                        nc.vector.tensor_tensor(
                            out=c_sb[:, ch], in0=t1[:], in1=t2[:], op=OP.add
                        )
                        tc_t = pwk.tile([8, W], f32, tag="tc_t")
                        nc.scalar.activation(tc_t[:], c_sb[:, ch], AF.Tanh)
                        nc.vector.tensor_tensor(
                            out=h_t[:, ch],
                            in0=sig[:, 2 * W : 3 * W],
                            in1=tc_t[:],
                            op=OP.mult,
                        )
                        # transpose this chunk's h back to [128, 8] k-tiles
                        for jj in range(KPC):
                            j = cki * KPC + jj
                            nc.tensor.transpose(
                                hps[:, j * 8 : (j + 1) * 8],
                                h_t[:, j * 128 : (j + 1) * 128],
                                eye8,
                            )
                            nt = phT.tile([128, 8], mm_dt, tag="hT")
                            if j % 2 == 0:
                                nc.scalar.copy(nt[:], hps[:, j * 8 : (j + 1) * 8])
                            else:
                                nc.vector.tensor_copy(
                                    nt[:], hps[:, j * 8 : (j + 1) * 8]
                                )
                            new_hT[j] = nt
                    hT = new_hT
                    if ti == TB - 1:
                        nc.sync.dma_start(
                            hs_v[:, t_ - TB + 1 : t_ + 1, :],
                            hbuf[:].rearrange("b (s h) -> b s h", s=TB),
                        )

            # ---------------- Phase C: FC + log_softmax ----------------
            with tc.tile_pool(name="pc_sb", bufs=3) as pc, tc.tile_pool(
                name="pc_keep", bufs=1
            ) as pck, tc.tile_pool(
                name="pc_ps", bufs=2, space="PSUM"
            ) as pc_ps, tc.tile_pool(
                name="pc_lps", bufs=2, space="PSUM"
            ) as pc_lps:
                logit_sb = pck.tile([128, ntiles * TAGS], f32)
                e_sb = pck.tile([128, ntiles * TAGS], f32)
                for k in range(ntiles):
                    hsk = pc.tile([128, HID], f32, tag="hsk")
                    nc.sync.dma_start(hsk[:], hs_dram[k * 128 : (k + 1) * 128, :])
                    lps = pc_lps.tile([128, TAGS], f32, tag="lps")
                    for j in range(HID // 128):
                        tp = pc_ps.tile([128, 128], f32, tag="trps")
                        nc.tensor.transpose(
                            tp[:], hsk[:, j * 128 : (j + 1) * 128], id128[:]
                        )
                        hsT = pc.tile([128, 128], f32, tag="hsT")
                        nc.vector.tensor_copy(hsT[:], tp[:])
                        nc.tensor.matmul(
                            lps[:],
                            lhsT=hsT[:],
                            rhs=fcw_sb[j][:],
                            start=(j == 0),
                            stop=False,
                        )
                    nc.tensor.matmul(
                        lps[:],
                        lhsT=ones_r[:1, :],
                        rhs=fcb_sb[:1, :],
                        start=False,
                        stop=True,
                    )
                    ks = slice(k * TAGS, (k + 1) * TAGS)
                    nc.scalar.activation(e_sb[:, ks], lps[:], AF.Exp)
                    nc.vector.tensor_copy(logit_sb[:, ks], lps[:])
                ssum = pck.tile([128, ntiles], f32)
                ev = e_sb[:].rearrange("p (k t) -> p k t", t=TAGS)
                nc.vector.tensor_reduce(
                    out=ssum[:], in_=ev, axis=mybir.AxisListType.X, op=OP.add
                )
                lsum = pck.tile([128, ntiles], f32)
                nc.scalar.activation(lsum[:], ssum[:], AF.Ln)
                for k in range(ntiles):
                    ks = slice(k * TAGS, (k + 1) * TAGS)
                    o_sb = pc.tile([128, TAGS], f32, tag="o_sb")
                    nc.vector.tensor_scalar(
                        out=o_sb[:],
                        in0=logit_sb[:, ks],
                        scalar1=lsum[:, k : k + 1],
                        scalar2=None,
                        op0=OP.subtract,
                    )
                    nc.sync.dma_start(outd[k * 128 : (k + 1) * 128, :], o_sb[:])

    if compile_nc:
        nc.compile()
    return nc, tok


def _gate_perm(nchunks):
    """Column permutation of the 2048 gate axis: chunk-major [i_c|f_c|o_c|g_c].
    Reference gate row blocks: i=0:512, f=512:1024, g=1024:1536, o=1536:2048."""
    w = HID // nchunks
    perm = []
    for c in range(nchunks):
        for blk in (0, 512, 1536, 1024):  # i, f, o, g
            perm.extend(range(blk + c * w, blk + (c + 1) * w))
    return np.array(perm)


def _prep_inputs(x, emb, W_ih, W_hh, b_hh, fc_W, fc_b, nsteps, w_dt_name,
                 nchunks=NCHUNKS):
    x = np.asarray(x)
    emb = np.ascontiguousarray(np.asarray(emb, dtype=np.float32))
    W_ih = np.asarray(W_ih, dtype=np.float32)
    W_hh = np.asarray(W_hh, dtype=np.float32)
    b_hh = np.asarray(b_hh, dtype=np.float32)
    fc_W = np.asarray(fc_W, dtype=np.float32)
    fc_b = np.asarray(fc_b, dtype=np.float32)

    w_np = np.float32
    if w_dt_name == "bfloat16":
        import ml_dtypes

        w_np = ml_dtypes.bfloat16

    perm = _gate_perm(nchunks)
    wih_p = np.ascontiguousarray(W_ih[perm, :].T, dtype=np.float32)
    whh_p = np.ascontiguousarray(W_hh[perm, :].T).astype(w_np)
    bhh_p = np.ascontiguousarray(b_hh[perm].reshape(1, H4), dtype=np.float32)
    fcw_t = np.ascontiguousarray(fc_W.T, dtype=np.float32)
    fcb_r = np.ascontiguousarray(fc_b.reshape(1, TAGS), dtype=np.float32)

    tok = BL * nsteps
    in_maps = []
    for c in range(NCORES):
        xc = x[c * BL : (c + 1) * BL, :nsteps].astype(np.int32).reshape(tok)
        xdev = np.ascontiguousarray(xc.reshape(tok // 128, 128).T)
        in_maps.append(
            {
                "xidx": xdev,
                "emb": emb,
                "wih": wih_p,
                "whh": whh_p,
                "bhh": bhh_p,
                "fcw": fcw_t,
                "fcb": fcb_r,
            }
        )
    return in_maps


def _get_runner(nsteps=S, w_dt_name="float32", repeat=1, nchunks=NCHUNKS):
    """Returns (run_fn, nc, put_inputs, run_dev)."""
    key = (nsteps, w_dt_name, repeat, nchunks)
    if key in _CACHE:
        return _CACHE[key]

    import jax
    from jax.sharding import Mesh, PartitionSpec, NamedSharding
    from jax.experimental.shard_map import shard_map
    from concourse import bass2jax, mybir

    nckey = ("nc",) + key
    if nckey not in _CACHE:
        _CACHE[nckey] = _build(nsteps, w_dt_name, repeat, nchunks)
    nc, tok = _CACHE[nckey]
    bass2jax.install_neuronx_cc_hook()

    partition_name = nc.partition_id_tensor.name if nc.partition_id_tensor else None
    in_names, out_names, out_avals, zero_shapes = [], [], [], []
    for alloc in nc.m.functions[0].allocations:
        if not isinstance(alloc, mybir.MemoryLocationSet):
            continue
        name = alloc.memorylocations[0].name
        if alloc.kind == "ExternalInput":
            if name != partition_name:
                in_names.append(name)
        elif alloc.kind == "ExternalOutput":
            shape = tuple(alloc.tensor_shape)
            dtype = mybir.dt.np(alloc.dtype)
            out_names.append(name)
            out_avals.append(jax.core.ShapedArray(shape, dtype))
            zero_shapes.append((shape, dtype))
    n_params = len(in_names)
    n_outs = len(out_avals)
    all_in_names = in_names + out_names + ([partition_name] if partition_name else [])
    donate = tuple(range(n_params, n_params + n_outs))

    def _body(*args):
        operands = list(args)
        if partition_name is not None:
            operands.append(bass2jax.partition_id_tensor())
        return tuple(
            bass2jax._bass_exec_p.bind(
                *operands,
                out_avals=tuple(out_avals),
                in_names=tuple(all_in_names),
                out_names=tuple(out_names),
                lowering_input_output_aliases=(),
                sim_require_finite=True,
                sim_require_nnan=True,
                nc=nc,
            )
        )

    devices = jax.devices()[:NCORES]
    mesh = Mesh(np.asarray(devices), ("core",))
    sharded = jax.jit(
        shard_map(
            _body,
            mesh=mesh,
            in_specs=(PartitionSpec("core"),) * (n_params + n_outs),
            out_specs=(PartitionSpec("core"),) * n_outs,
            check_rep=False,
        ),
        donate_argnums=donate,
        keep_unused=True,
    )
    shard = NamedSharding(mesh, PartitionSpec("core"))

    def put_inputs(in_maps):
        concat_in = [
            np.concatenate([np.asarray(m[nm]) for m in in_maps], axis=0)
            for nm in in_names
        ]
        dev_in = [jax.device_put(a, shard) for a in concat_in]
        jax.block_until_ready(dev_in)
        return dev_in

    def run_dev(dev_in):
        import time as _time

        concat_zeros = [
            jax.device_put(np.zeros((NCORES * s[0], *s[1:]), d), shard)
            for (s, d) in zero_shapes
        ]
        jax.block_until_ready(concat_zeros)
        t0 = _time.time()
        out_arrs = sharded(*dev_in, *concat_zeros)
        jax.block_until_ready(out_arrs)
        dt = _time.time() - t0
        return out_arrs, dt

    def run_fn(in_maps):
        out_arrs, _ = run_dev(put_inputs(in_maps))
        return [
            {
                nm: np.asarray(out_arrs[i]).reshape(NCORES, *out_avals[i].shape)[c]
                for i, nm in enumerate(out_names)
            }
            for c in range(NCORES)
        ]

    _CACHE[key] = (run_fn, nc, put_inputs, run_dev)
    return _CACHE[key]


W_DT_NAME = "float32"


def kernel(x, emb, W_ih, W_hh, b_hh, fc_W, fc_b):
    from concourse.bass_utils import run_bass_kernel_spmd

    key = ("nc", S, W_DT_NAME, 1, NCHUNKS)
    if key not in _CACHE:
        _CACHE[key] = _build(S, W_DT_NAME, 1, NCHUNKS)
    nc, _tok = _CACHE[key]
    in_maps = _prep_inputs(x, emb, W_ih, W_hh, b_hh, fc_W, fc_b, S, W_DT_NAME)
    res = run_bass_kernel_spmd(nc, in_maps, core_ids=list(range(NCORES)))
    out = np.stack(
        [res.results[c]["out"].reshape(BL, S, TAGS) for c in range(NCORES)]
    )
    return out.reshape(B, S, TAGS).astype(np.float32)


# revision 19
# speedup vs baseline: 4557.4837x; 1.2733x over previous
"""Trainium2 Bass kernel for nn_BayesianDropoutLSTM_42468636623062.

Strategy (8 NeuronCores, data-parallel over batch):
  - Each core owns B/8 = 8 batch rows; weights replicated.
  - Phase A: embedding gather (indirect DMA) + px = xe @ W_ih.T + b_hh
    precompute (PE matmul, bias folded in as a rank-1 matmul), px -> HBM.
  - Phase B: 512 sequential LSTM steps. The 2048 gate columns are permuted
    host-side into hidden-chunk-major order [i_c|f_c|o_c|g_c] per chunk so
    each chunk's sigmoid is one ACT op and chunks pipeline: chunk c's
    elementwise chain overlaps the PE matmuls of the other chunk / next
    step. gates accumulate in PSUM: px_t injected via an eye(8) matmul,
    then h_{t-1} @ W_hh.T as K-chunked matmuls with h.T as the stationary
    operand (h is PE-transposed back to [128, 8] chunks each step).
    All big matmuls use float32r (1 cycle/row vs 4 for fp32).
  - Phase C: logits = hs @ fc_W.T + fc_b (PE, bias rank-1), then
    log_softmax = x - ln(sum(exp(x))) with exp/ln on ACT.
"""

import numpy as np

VOCAB, TAGS, EMB, HID = 100000, 48, 256, 512
B, S = 64, 512
H4 = 4 * HID
NCORES = 8
BL = B // NCORES  # 8 local batch rows per core

NCHUNKS = 4  # hidden-dim pipeline chunks in phase B (1, 2, or 4)

_CACHE = {}
_BUFS = {}


def _build(nsteps=S, w_dt_name="float32", repeat=1, nchunks=NCHUNKS, abl=(),
           compile_nc=True):
    """Build + compile the per-core Bass program. Returns (nc, tok)."""
    abl = set(abl)
    import concourse.bass as bass
    import concourse.tile as tile
    from concourse import bacc, mybir
    from concourse.masks import make_identity
    from contextlib import ExitStack

    f32 = mybir.dt.float32
    f32r = mybir.dt.float32r
    i32 = mybir.dt.int32
    w_dt = getattr(mybir.dt, w_dt_name)
    AF = mybir.ActivationFunctionType
    OP = mybir.AluOpType

    # matmul storage dtype for the big MMs: float32r streams 1 cycle/row on
    # the PE (vs 4 for fp32) at N>=256. The BIR verifier requires f32r matmul
    # operands to be produced rounded, so the feeding tiles are natively f32r
    # and weights are converted once on-device via DVE copies.
    mm_dt = f32r if w_dt == f32 else w_dt

    tok = BL * nsteps
    ntiles = tok // 128
    assert tok % 128 == 0
    W = HID // nchunks       # hidden units per chunk
    GW = 4 * W               # gate columns per chunk
    KCH = HID // 128         # K chunks (4)
    KPC = KCH // nchunks     # K chunks produced per hidden chunk

    nc = bacc.Bacc(
        "TRN2",
        target_bir_lowering=False,
        debug=False,
        enable_asserts=True,
        num_devices=NCORES,
    )

    xidx = nc.dram_tensor("xidx", [128, ntiles], i32, kind="ExternalInput")
    emb = nc.dram_tensor("emb", [VOCAB, EMB], f32, kind="ExternalInput")
    wih = nc.dram_tensor("wih", [EMB, H4], f32, kind="ExternalInput")  # W_ih.T perm
    whh = nc.dram_tensor("whh", [HID, H4], w_dt, kind="ExternalInput")  # W_hh.T perm
    bhh = nc.dram_tensor("bhh", [1, H4], f32, kind="ExternalInput")
    fcw = nc.dram_tensor("fcw", [HID, TAGS], f32, kind="ExternalInput")  # fc_W.T
    fcb = nc.dram_tensor("fcb", [1, TAGS], f32, kind="ExternalInput")
    outd = nc.dram_tensor("out", [tok, TAGS], f32, kind="ExternalOutput")

    with tile.TileContext(nc) as tc, ExitStack() as ctx:
        const_pool = ctx.enter_context(tc.tile_pool(name="const", bufs=1))
        dram_pool = ctx.enter_context(tc.tile_pool(name="dram", bufs=1, space="DRAM"))

        px_dram = dram_pool.tile([tok, H4], mm_dt)
        hs_dram = dram_pool.tile([tok, HID], f32)

        id128 = const_pool.tile([128, 128], f32)
        make_identity(nc, id128[:])
        ones_r = const_pool.tile([1, 128], f32)
        nc.vector.memset(ones_r[:], 1.0)
        x_sb = const_pool.tile([128, ntiles], i32)
        nc.sync.dma_start(x_sb[:], xidx[:])
        bhh_sb = const_pool.tile([1, H4], f32)
        nc.sync.dma_start(bhh_sb[:], bhh[:])
        fcb_sb = const_pool.tile([1, TAGS], f32)
        nc.sync.dma_start(fcb_sb[:], fcb[:])
        eye8 = id128[:8, :8]
        eye8w_t = const_pool.tile([8, 8], mm_dt)
        nc.vector.tensor_copy(eye8w_t[:], eye8)
        eye8w = eye8w_t[:]
        ones_rr = const_pool.tile([1, 128], mm_dt)
        nc.vector.tensor_copy(ones_rr[:], ones_r[:])
        bhh_rr = const_pool.tile([1, H4], mm_dt)
        nc.vector.tensor_copy(bhh_rr[:], bhh_sb[:])

        wih_sb = []
        whh_sb = []
        with tc.tile_pool(name="wstage", bufs=2) as pstage:
            for j in range(EMB // 128):
                st = pstage.tile([128, H4], f32, tag="st")
                nc.sync.dma_start(st[:], wih[j * 128 : (j + 1) * 128, :])
                t = const_pool.tile([128, H4], mm_dt, tag=f"wih{j}")
                nc.vector.tensor_copy(t[:], st[:])
                wih_sb.append(t)
            for j in range(HID // 128):
                if w_dt == f32:
                    st = pstage.tile([128, H4], f32, tag="st")
                    nc.sync.dma_start(st[:], whh[j * 128 : (j + 1) * 128, :])
                    t = const_pool.tile([128, H4], mm_dt, tag=f"whh{j}")
                    nc.vector.tensor_copy(t[:], st[:])
                else:
                    t = const_pool.tile([128, H4], w_dt, tag=f"whh{j}")
                    nc.sync.dma_start(t[:], whh[j * 128 : (j + 1) * 128, :])
                whh_sb.append(t)
        fcw_sb = []
        for j in range(HID // 128):
            t = const_pool.tile([128, TAGS], f32, tag=f"fcw{j}")
            nc.sync.dma_start(t[:], fcw[j * 128 : (j + 1) * 128, :])
            fcw_sb.append(t)

        px_v = px_dram[:].rearrange("(b s) g -> b s g", b=BL)
        hs_v = hs_dram[:].rearrange("(b s) h -> b s h", b=BL)

        for _rep in range(repeat):
            # ---------------- Phase A: gather + px precompute ----------------
            with tc.tile_pool(name="pa_sb", bufs=3) as pa, tc.tile_pool(
                name="pa_ps", bufs=2, space="PSUM"
            ) as pa_ps, tc.tile_pool(name="pa_pxps", bufs=1, space="PSUM") as pa_pxps:
                for k in range(ntiles):
                    xe = pa.tile([128, EMB], f32, tag="xe")
                    nc.gpsimd.indirect_dma_start(
                        out=xe[:],
                        out_offset=None,
                        in_=emb[:],
                        in_offset=bass.IndirectOffsetOnAxis(
                            ap=x_sb[:, k : k + 1], axis=0
                        ),
                    )
                    xeT = []
                    for j in range(EMB // 128):
                        tp = pa_ps.tile([128, 128], f32, tag="trps")
                        nc.tensor.transpose(
                            tp[:], xe[:, j * 128 : (j + 1) * 128], id128[:]
                        )
                        xt = pa.tile([128, 128], mm_dt, tag=f"xeT{j}")
                        nc.vector.tensor_copy(xt[:], tp[:])
                        xeT.append(xt)
                    pxps = pa_pxps.tile([128, H4], f32, tag="pxps")
                    for bank in range(4):
                        bs = slice(bank * 512, (bank + 1) * 512)
                        for j in range(EMB // 128):
                            nc.tensor.matmul(
                                pxps[:, bs],
                                lhsT=xeT[j][:],
                                rhs=wih_sb[j][:, bs],
                                start=(j == 0),
                                stop=False,
                            )
                        nc.tensor.matmul(
                            pxps[:, bs],
                            lhsT=ones_rr[:1, :],
                            rhs=bhh_rr[:1, bs],
                            start=False,
                            stop=True,
                        )
                    px_sb = pa.tile([128, H4], mm_dt, tag="px_sb")
                    nc.vector.tensor_copy(px_sb[:], pxps[:])
                    nc.sync.dma_start(px_dram[k * 128 : (k + 1) * 128, :], px_sb[:])

            # ---------------- Phase B: recurrence ----------------
            _bf = _BUFS
            with tc.tile_pool(name="pb_state", bufs=1) as pst, tc.tile_pool(
                name="pb_hT", bufs=_bf.get("hT", 10)
            ) as phT, tc.tile_pool(name="pb_px", bufs=_bf.get("px", 2)) as ppx, tc.tile_pool(
                name="pb_wk", bufs=_bf.get("wk", 4)
            ) as pwk, tc.tile_pool(
                name="pb_ps", bufs={1: 1, 2: 3, 4: 6}[nchunks],
                space="PSUM",
            ) as pps, tc.tile_pool(
                name="pb_hps", bufs=2, space="PSUM"
            ) as phps:
                c_sb = pst.tile([8, HID], f32)
                nc.vector.memset(c_sb[:], 0.0)
                KCHn = HID // 128
                hT = []
                for j in range(KCHn):
                    t = phT.tile([128, 8], mm_dt, tag="hT")
                    mv = t[:].bitcast(f32) if mm_dt == f32r else t[:]
                    nc.vector.memset(mv, 0.0)
                    hT.append(t)

                nbank = GW // 512  # PSUM banks per chunk psum tile
                TB = 4  # steps per batched px-load / hs-store DMA
                assert nsteps % TB == 0
                px_t8 = None
                hbuf = None
                for t_ in range(nsteps):
                    ti = t_ % TB
                    if ti == 0:
                        px_t8 = ppx.tile([8, TB * H4], mm_dt, tag="px_t")
                        nc.sync.dma_start(
                            px_t8[:].rearrange("b (s g) -> b s g", s=TB),
                            px_v[:, t_ : t_ + TB, :],
                        )
                        hbuf = pwk.tile([8, TB * HID], f32, tag="h_t")
                    h_t = hbuf[:, ti * HID : (ti + 1) * HID]
                    new_hT = [None] * KCHn
                    hps = phps.tile([128, 8 * KCHn], f32, tag="hps")
                    for cki in range(nchunks):
                        gps = pps.tile([8, GW], f32, tag="gps")
                        for bank in range(nbank):
                            bs = slice(bank * 512, (bank + 1) * 512)
                            bsg = slice(
                                cki * GW + bank * 512, cki * GW + (bank + 1) * 512
                            )
                            bsg_px = slice(
                                ti * H4 + cki * GW + bank * 512,
                                ti * H4 + cki * GW + (bank + 1) * 512,
                            )
                            nc.tensor.matmul(
                                gps[:, bs],
                                lhsT=eye8w,
                                rhs=px_t8[:, bsg_px],
                                start=True,
                                stop=False,
                            )
                            if "no_hmm" in abl:
                                continue
                            for j in range(KCHn):
                                nc.tensor.matmul(
                                    gps[:, bs],
                                    lhsT=hT[j][:],
                                    rhs=whh_sb[j][:, bsg],
                                    start=False,
                                    stop=(j == KCHn - 1),
                                )
                        # chunk gate layout: [i (W) | f (W) | o (W) | g (W)]
                        sig = pwk.tile([8, 3 * W], f32, tag="sig")
                        nc.scalar.activation(sig[:], gps[:, 0 : 3 * W], AF.Sigmoid)
                        g_t = pwk.tile([8, W], f32, tag="g_t")
                        nc.scalar.activation(g_t[:], gps[:, 3 * W : 4 * W], AF.Tanh)
                        ch = slice(cki * W, (cki + 1) * W)  # hidden slice
                        t1 = pwk.tile([8, W], f32, tag="t1")
                        nc.vector.tensor_tensor(
                            out=t1[:], in0=sig[:, 0:W], in1=g_t[:], op=OP.mult
                        )
                        t2 = pwk.tile([8, W], f32, tag="t2")
                        eng_t2 = nc.gpsimd if "gp_t2" in abl else nc.vector
                        eng_t2.tensor_tensor(
                            out=t2[:],
                            in0=sig[:, W : 2 * W],
                            in1=c_sb[:, ch],
                            op=OP.mult,
                        )
                        nc.vector.tensor_tensor(
                            out=c_sb[:, ch], in0=t1[:], in1=t2[:], op=OP.add
                        )
                        tc_t = pwk.tile([8, W], f32, tag="tc_t")
                        nc.scalar.activation(tc_t[:], c_sb[:, ch], AF.Tanh)
                        nc.vector.tensor_tensor(
                            out=h_t[:, ch],
                            in0=sig[:, 2 * W : 3 * W],
                            in1=tc_t[:],
                            op=OP.mult,
                        )
                        # transpose this chunk's h back to [128, 8] k-tiles
                        for jj in range(KPC):
                            j = cki * KPC + jj
                            nc.tensor.transpose(
                                hps[:, j * 8 : (j + 1) * 8],
                                h_t[:, j * 128 : (j + 1) * 128],
                                eye8,
                            )
                            nt = phT.tile([128, 8], mm_dt, tag="hT")
                            if j % 2 == 0:
                                nc.scalar.copy(nt[:], hps[:, j * 8 : (j + 1) * 8])
                            else:
                                nc.vector.tensor_copy(
                                    nt[:], hps[:, j * 8 : (j + 1) * 8]
                                )
                            new_hT[j] = nt
                    hT = new_hT
                    if ti == TB - 1:
                        nc.sync.dma_start(
                            hs_v[:, t_ - TB + 1 : t_ + 1, :],
                            hbuf[:].rearrange("b (s h) -> b s h", s=TB),
                        )

            # ---------------- Phase C: FC + log_softmax ----------------
            with tc.tile_pool(name="pc_sb", bufs=3) as pc, tc.tile_pool(
                name="pc_keep", bufs=1
            ) as pck, tc.tile_pool(
                name="pc_ps", bufs=2, space="PSUM"
            ) as pc_ps, tc.tile_pool(
                name="pc_lps", bufs=2, space="PSUM"
            ) as pc_lps:
                logit_sb = pck.tile([128, ntiles * TAGS], f32)
                e_sb = pck.tile([128, ntiles * TAGS], f32)
                for k in range(ntiles):
                    hsk = pc.tile([128, HID], f32, tag="hsk")
                    nc.sync.dma_start(hsk[:], hs_dram[k * 128 : (k + 1) * 128, :])
                    lps = pc_lps.tile([128, TAGS], f32, tag="lps")
                    for j in range(HID // 128):
                        tp = pc_ps.tile([128, 128], f32, tag="trps")
                        nc.tensor.transpose(
                            tp[:], hsk[:, j * 128 : (j + 1) * 128], id128[:]
                        )
                        hsT = pc.tile([128, 128], f32, tag="hsT")
                        nc.vector.tensor_copy(hsT[:], tp[:])
                        nc.tensor.matmul(
                            lps[:],
                            lhsT=hsT[:],
                            rhs=fcw_sb[j][:],
                            start=(j == 0),
                            stop=False,
                        )
                    nc.tensor.matmul(
                        lps[:],
                        lhsT=ones_r[:1, :],
                        rhs=fcb_sb[:1, :],
                        start=False,
                        stop=True,
                    )
                    ks = slice(k * TAGS, (k + 1) * TAGS)
                    nc.scalar.activation(e_sb[:, ks], lps[:], AF.Exp)
                    nc.vector.tensor_copy(logit_sb[:, ks], lps[:])
                ssum = pck.tile([128, ntiles], f32)
                ev = e_sb[:].rearrange("p (k t) -> p k t", t=TAGS)
                nc.vector.tensor_reduce(
                    out=ssum[:], in_=ev, axis=mybir.AxisListType.X, op=OP.add
                )
                lsum = pck.tile([128, ntiles], f32)
                nc.scalar.activation(lsum[:], ssum[:], AF.Ln)
                for k in range(ntiles):
                    ks = slice(k * TAGS, (k + 1) * TAGS)
                    o_sb = pc.tile([128, TAGS], f32, tag="o_sb")
                    nc.vector.tensor_scalar(
                        out=o_sb[:],
                        in0=logit_sb[:, ks],
                        scalar1=lsum[:, k : k + 1],
                        scalar2=None,
                        op0=OP.subtract,
                    )
                    nc.sync.dma_start(outd[k * 128 : (k + 1) * 128, :], o_sb[:])

    if compile_nc:
        nc.compile()
    return nc, tok


def _gate_perm(nchunks):
    """Column permutation of the 2048 gate axis: chunk-major [i_c|f_c|o_c|g_c].
    Reference gate row blocks: i=0:512, f=512:1024, g=1024:1536, o=1536:2048."""
    w = HID // nchunks
    perm = []
    for c in range(nchunks):
        for blk in (0, 512, 1536, 1024):  # i, f, o, g
            perm.extend(range(blk + c * w, blk + (c + 1) * w))
    return np.array(perm)


def _prep_inputs(x, emb, W_ih, W_hh, b_hh, fc_W, fc_b, nsteps, w_dt_name,
                 nchunks=NCHUNKS):
    x = np.asarray(x)
    emb = np.ascontiguousarray(np.asarray(emb, dtype=np.float32))
    W_ih = np.asarray(W_ih, dtype=np.float32)
    W_hh = np.asarray(W_hh, dtype=np.float32)
    b_hh = np.asarray(b_hh, dtype=np.float32)
    fc_W = np.asarray(fc_W, dtype=np.float32)
    fc_b = np.asarray(fc_b, dtype=np.float32)

    w_np = np.float32
    if w_dt_name == "bfloat16":
        import ml_dtypes

        w_np = ml_dtypes.bfloat16

    perm = _gate_perm(nchunks)
    wih_p = np.ascontiguousarray(W_ih[perm, :].T, dtype=np.float32)
    whh_p = np.ascontiguousarray(W_hh[perm, :].T).astype(w_np)
    bhh_p = np.ascontiguousarray(b_hh[perm].reshape(1, H4), dtype=np.float32)
    fcw_t = np.ascontiguousarray(fc_W.T, dtype=np.float32)
    fcb_r = np.ascontiguousarray(fc_b.reshape(1, TAGS), dtype=np.float32)

    tok = BL * nsteps
    in_maps = []
    for c in range(NCORES):
        xc = x[c * BL : (c + 1) * BL, :nsteps].astype(np.int32).reshape(tok)
        xdev = np.ascontiguousarray(xc.reshape(tok // 128, 128).T)
        in_maps.append(
            {
                "xidx": xdev,
                "emb": emb,
                "wih": wih_p,
                "whh": whh_p,
                "bhh": bhh_p,
                "fcw": fcw_t,
                "fcb": fcb_r,
            }
        )
    return in_maps


def _get_runner(nsteps=S, w_dt_name="float32", repeat=1, nchunks=NCHUNKS):
    """Returns (run_fn, nc, put_inputs, run_dev)."""
    key = (nsteps, w_dt_name, repeat, nchunks)
    if key in _CACHE:
        return _CACHE[key]

    import jax
    from jax.sharding import Mesh, PartitionSpec, NamedSharding
    from jax.experimental.shard_map import shard_map
    from concourse import bass2jax, mybir

    nckey = ("nc",) + key
    if nckey not in _CACHE:
        _CACHE[nckey] = _build(nsteps, w_dt_name, repeat, nchunks)
    nc, tok = _CACHE[nckey]
    bass2jax.install_neuronx_cc_hook()

    partition_name = nc.partition_id_tensor.name if nc.partition_id_tensor else None
    in_names, out_names, out_avals, zero_shapes = [], [], [], []
    for alloc in nc.m.functions[0].allocations:
        if not isinstance(alloc, mybir.MemoryLocationSet):
            continue
        name = alloc.memorylocations[0].name
        if alloc.kind == "ExternalInput":
            if name != partition_name:
                in_names.append(name)
        elif alloc.kind == "ExternalOutput":
            shape = tuple(alloc.tensor_shape)
            dtype = mybir.dt.np(alloc.dtype)
            out_names.append(name)
            out_avals.append(jax.core.ShapedArray(shape, dtype))
            zero_shapes.append((shape, dtype))
    n_params = len(in_names)
    n_outs = len(out_avals)
    all_in_names = in_names + out_names + ([partition_name] if partition_name else [])
    donate = tuple(range(n_params, n_params + n_outs))

    def _body(*args):
        operands = list(args)
        if partition_name is not None:
            operands.append(bass2jax.partition_id_tensor())
        return tuple(
            bass2jax._bass_exec_p.bind(
                *operands,
                out_avals=tuple(out_avals),
                in_names=tuple(all_in_names),
                out_names=tuple(out_names),
                lowering_input_output_aliases=(),
                sim_require_finite=True,
                sim_require_nnan=True,
                nc=nc,
            )
        )

    devices = jax.devices()[:NCORES]
    mesh = Mesh(np.asarray(devices), ("core",))
    sharded = jax.jit(
        shard_map(
            _body,
            mesh=mesh,
            in_specs=(PartitionSpec("core"),) * (n_params + n_outs),
            out_specs=(PartitionSpec("core"),) * n_outs,
            check_rep=False,
        ),
        donate_argnums=donate,
        keep_unused=True,
    )
    shard = NamedSharding(mesh, PartitionSpec("core"))

    def put_inputs(in_maps):
        concat_in = [
            np.concatenate([np.asarray(m[nm]) for m in in_maps], axis=0)
            for nm in in_names
        ]
        dev_in = [jax.device_put(a, shard) for a in concat_in]
        jax.block_until_ready(dev_in)
        return dev_in

    def run_dev(dev_in):
        import time as _time

        concat_zeros = [
            jax.device_put(np.zeros((NCORES * s[0], *s[1:]), d), shard)
            for (s, d) in zero_shapes
        ]
        jax.block_until_ready(concat_zeros)
        t0 = _time.time()
        out_arrs = sharded(*dev_in, *concat_zeros)
        jax.block_until_ready(out_arrs)
        dt = _time.time() - t0
        return out_arrs, dt

    def run_fn(in_maps):
        out_arrs, _ = run_dev(put_inputs(in_maps))
        return [
            {
                nm: np.asarray(out_arrs[i]).reshape(NCORES, *out_avals[i].shape)[c]
                for i, nm in enumerate(out_names)
            }
            for c in range(NCORES)
        ]

    _CACHE[key] = (run_fn, nc, put_inputs, run_dev)
    return _CACHE[key]


W_DT_NAME = "float32"


def kernel(x, emb, W_ih, W_hh, b_hh, fc_W, fc_b):
    from concourse.bass_utils import run_bass_kernel_spmd

    key = ("nc", S, W_DT_NAME, 1, NCHUNKS)
    if key not in _CACHE:
        _CACHE[key] = _build(S, W_DT_NAME, 1, NCHUNKS)
    nc, _tok = _CACHE[key]
    in_maps = _prep_inputs(x, emb, W_ih, W_hh, b_hh, fc_W, fc_b, S, W_DT_NAME)
    res = run_bass_kernel_spmd(nc, in_maps, core_ids=list(range(NCORES)))
    out = np.stack(
        [res.results[c]["out"].reshape(BL, S, TAGS) for c in range(NCORES)]
    )
    return out.reshape(B, S, TAGS).astype(np.float32)
